# revision 33
# baseline (speedup 1.0000x reference)
"""DendriticMatcherBank Trainium2 kernel (v3.1, streaming).

Math (per (b, k)):
    arrivals[b,k,t] = spike[b,t] + delays[k,t]
    tref[b,k]       = max_t arrivals
    psp             = exp((arrivals - tref)/40)
    I               = w * psp
    v_t             = 0.95 v_{t-1} + 0.05 I_t
    matcher[b,k]    = sum_t t*exp(v_t) / sum_t exp(v_t)  (softmax shift skipped: |v| <= ~1.5)

Sharding: B split across 8 cores (2048 rows each); (K,T) params replicated.

On-chip layout: partitions = 128 consecutive b; free = (k-chunk of G, t).

Engine assignment (per-core full passes over 16.8M elements):
  GPSIMD: arrivals = spike+delays;  I = (alpha w) * psp
  ACT:    psp = exp(arr/40 - tref/40) (per-k ops; bias is per-partition);
          e = exp(v); e2 = exp(v2)  (wide ops)
  DVE:    tref = rmax(arrivals); v = EMA scan; den = rsum(e); num = rsum(e2)
  PE:     v2 = v + ln(t) into PSUM (identity + rank-1 matmuls; PE idle otherwise)
The EMA runs as one tensor_tensor_scan along the flattened (g t) free dim,
with the decay operand zeroed at each segment start to reset between k's.
"""

import numpy as np

import concourse.bass as bass
import concourse.tile as tile
from concourse import mybir
from concourse.bass_utils import run_bass_kernel_spmd

B, K, T = 16384, 64, 128
N_CORES = 8
BL = B // N_CORES          # 2048 b-rows per core
PB = 128                   # b-rows per partition block
NB = BL // PB              # 16 b-blocks
G = 8                      # k's processed per chunk
NKC = K // G               # 8 k-chunks
W = G * T                  # 1024 free width per chunk
SLOT4 = 4                  # matmul slots per PSUM bank (4*T = 512 fp32 = 1 bank)
TAU_PSP = 40.0
ALPHA = 0.05               # DT / TAU_MEM
DECAY = 1.0 - ALPHA
F32 = mybir.dt.float32


def _split_multiwait(nc, max_waits=1):
    """Walrus in this env encodes at most one sync-wait per instruction; Tile's
    kernel-tail drain waits on every DMA-HW sem at once. Hoist extra waits onto
    single-wait NoOps inserted just before the offending instruction."""
    for fn in nc.m.functions:
        try:
            items = list(fn.blocks.items())
        except Exception:
            items = list(enumerate(fn.blocks))
        for _, blk in items:
            newlist = []
            changed = False
            for ins in blk.instructions:
                si = ins.sync_info
                waits = list(si.on_wait) if (si is not None and si.on_wait) else []
                if len(waits) > max_waits:
                    head, keep = waits[:-max_waits], waits[-max_waits:]
                    for wi, wt in enumerate(head):
                        nop = mybir.InstNoOp(name=f"{ins.name}-w{wi}", ins=[], outs=[])
                        nop.engine = ins.engine
                        nop.sync_info = mybir.SyncInfo(on_wait=[wt], on_update=[])
                        newlist.append(nop)
                    ins.sync_info = mybir.SyncInfo(
                        on_wait=keep, on_update=list(si.on_update or [])
                    )
                    changed = True
                newlist.append(ins)
            if changed:
                blk.instructions = newlist


def _pbcast(ap, p):
    """Broadcast an AP across p partitions (step-0 leading dim). DMA-only."""
    return bass.AP(tensor=ap.tensor, offset=ap.offset, ap=[[0, p]] + list(ap.ap))


def _build_nc(nb=NB, nkc=NKC, work_bufs=4, perib_bufs=2, stats_bufs=12, arr_bufs=6,
              psum_bufs=4, prio_s0=0):
    bl = nb * PB
    nc = bass.Bass("TRN2", num_devices=N_CORES)

    spike = nc.dram_tensor("spike", (bl, T), F32, kind="ExternalInput")
    delays = nc.dram_tensor("delays", (K, T), F32, kind="ExternalInput")
    aw = nc.dram_tensor("aw", (K, T), F32, kind="ExternalInput")        # ALPHA * w
    decay = nc.dram_tensor("decay", (W,), F32, kind="ExternalInput")    # 0.95, 0 at seg starts
    lnt = nc.dram_tensor("lnt", (W,), F32, kind="ExternalInput")        # ln(t) tiled, -1e4 at t=0
    ident = nc.dram_tensor("ident", (PB, PB), F32, kind="ExternalInput")
    ones1 = nc.dram_tensor("ones1", (PB,), F32, kind="ExternalInput")

    arrivals = nc.dram_tensor("arrivals", (bl, K, T), F32, kind="ExternalOutput")
    psp = nc.dram_tensor("psp", (bl, K, T), F32, kind="ExternalOutput")
    matcher = nc.dram_tensor("matcher", (bl, K), F32, kind="ExternalOutput")

    delays_f = delays.rearrange("k t -> (k t)")
    aw_f = aw.rearrange("k t -> (k t)")
    ngrp = G // SLOT4          # 4-slot matmul groups per chunk

    with tile.TileContext(nc) as tc:
        with (
            tc.tile_pool(name="consts", bufs=1) as consts,
            tc.tile_pool(name="spk", bufs=1) as spk,
            tc.tile_pool(name="perib", bufs=perib_bufs) as perib,
            tc.tile_pool(name="work", bufs=work_bufs) as work,
            tc.tile_pool(name="arrp", bufs=arr_bufs) as arrp,
            tc.tile_pool(name="stats", bufs=stats_bufs) as stats,
            tc.tile_pool(name="psB", bufs=psum_bufs, space="PSUM") as psB,
        ):
            spike_t = []
            for ib in range(nb):
                st = spk.tile([PB, T], F32, tag=f"spike{ib}")
                nc.sync.dma_start(out=st, in_=spike[ib * PB:(ib + 1) * PB, :])
                spike_t.append(st)

            decay_t = consts.tile([PB, W], F32, tag="decay")
            nc.sync.dma_start(out=decay_t, in_=_pbcast(decay.ap(), PB))
            ident_t = consts.tile([PB, PB], F32, tag="ident")
            nc.sync.dma_start(out=ident_t, in_=ident[:, :])
            ones1_t = consts.tile([1, PB], F32, tag="ones1")
            nc.sync.dma_start(out=ones1_t, in_=_pbcast(ones1.ap(), 1))
            lnt4_t = consts.tile([1, SLOT4 * T], F32, tag="lnt4")
            nc.sync.dma_start(out=lnt4_t, in_=_pbcast(lnt.ap()[:SLOT4 * T], 1))

            dly_bc = consts.tile([PB, K, T], F32, tag="dly")
            aw_bc = consts.tile([PB, K, T], F32, tag="aw")
            for kc in range(nkc):
                nc.sync.dma_start(
                    out=dly_bc[:, kc * G:(kc + 1) * G, :],
                    in_=_pbcast(delays_f[kc * G * T:(kc + 1) * G * T], PB),
                )
                nc.sync.dma_start(
                    out=aw_bc[:, kc * G:(kc + 1) * G, :],
                    in_=_pbcast(aw_f[kc * G * T:(kc + 1) * G * T], PB),
                )

            NCI = nb * nkc

            def s0(ci):
                if prio_s0:
                    with tc.high_priority(offset=prio_s0):
                        return s0_body(ci)
                return s0_body(ci)

            def s0_body(ci):
                ib, kc = divmod(ci, nkc)
                b0 = ib * PB
                ksl = slice(kc * G, (kc + 1) * G)
                sap = spike_t[ib][:, :]
                spike_bc = bass.AP(
                    tensor=sap.tensor, offset=sap.offset,
                    ap=[sap.ap[0], [0, G], sap.ap[1]],
                )
                arr = arrp.tile([PB, G, T], F32, tag="arr", name=f"arr{ci}")
                nc.gpsimd.tensor_add(
                    out=arr, in0=spike_bc, in1=dly_bc[:, ksl, :]
                )
                nc.sync.dma_start(out=arrivals[b0:b0 + PB, ksl, :], in_=arr)
                tref_c = stats.tile([PB, G], F32, tag="trefc", name=f"trefc{ci}")
                nc.vector.reduce_max(
                    out=tref_c, in_=arr, axis=mybir.AxisListType.X, negate=True,
                )
                ntref = stats.tile([PB, G], F32, tag="ntref", name=f"ntref{ci}")
                nc.scalar.mul(out=ntref, in_=tref_c, mul=1.0 / TAU_PSP)
                state[ci]["arr"] = arr
                state[ci]["ntref"] = ntref

            def s1(ci):
                ib, kc = divmod(ci, nkc)
                b0 = ib * PB
                ksl = slice(kc * G, (kc + 1) * G)
                arr = state[ci]["arr"]
                ntref = state[ci]["ntref"]
                psp_t = work.tile([PB, G, T], F32, tag="psp", name=f"pspt{ci}")
                for j in range(G):
                    nc.scalar.activation(
                        out=psp_t[:, j, :], in_=arr[:, j, :],
                        func=mybir.ActivationFunctionType.Exp,
                        scale=1.0 / TAU_PSP,
                        bias=ntref[:, j:j + 1],
                    )
                nc.sync.dma_start(out=psp[b0:b0 + PB, ksl, :], in_=psp_t)
                i_s = work.tile([PB, G, T], F32, tag="is", name=f"is{ci}")
                nc.gpsimd.tensor_mul(out=i_s, in0=psp_t, in1=aw_bc[:, ksl, :])
                state[ci]["i_s"] = i_s

            def s2(ci):
                i_s = state[ci]["i_s"]
                v = work.tile([PB, G, T], F32, tag="v", name=f"v{ci}")
                nc.vector.tensor_tensor_scan(
                    out=v.rearrange("p g t -> p (g t)"),
                    data0=decay_t[:, :],
                    data1=i_s.rearrange("p g t -> p (g t)"),
                    initial=0.0,
                    op0=mybir.AluOpType.mult,
                    op1=mybir.AluOpType.add,
                )
                e = work.tile([PB, G, T], F32, tag="e", name=f"e{ci}")
                nc.scalar.activation(
                    out=e, in_=v, func=mybir.ActivationFunctionType.Exp,
                )
                v2p = psB.tile([PB, G, T], F32, tag="v2p", name=f"v2p{ci}")
                for g in range(ngrp):
                    sl = v2p[:, g * SLOT4:(g + 1) * SLOT4, :]
                    nc.tensor.matmul(
                        sl, ones1_t[:, :], lnt4_t[:1, :], start=True, stop=False,
                    )
                    nc.tensor.matmul(
                        sl, ident_t[:, :], v[:, g * SLOT4:(g + 1) * SLOT4, :],
                        start=False, stop=True,
                    )
                e2 = work.tile([PB, G, T], F32, tag="e2", name=f"e2{ci}")
                nc.scalar.activation(
                    out=e2, in_=v2p, func=mybir.ActivationFunctionType.Exp,
                )
                state[ci]["e"] = e
                state[ci]["e2"] = e2

            def s3(ci):
                ib, kc = divmod(ci, nkc)
                if kc == 0:
                    blk_state[ib] = {
                        "den": perib.tile([PB, K], F32, tag="denblk", name=f"denblk{ib}"),
                        "num": perib.tile([PB, K], F32, tag="numblk", name=f"numblk{ib}"),
                    }
                ksl = slice(kc * G, (kc + 1) * G)
                nc.vector.reduce_sum(
                    out=blk_state[ib]["den"][:, ksl], in_=state[ci]["e"],
                    axis=mybir.AxisListType.X,
                )

            def s4(ci):
                ib, kc = divmod(ci, nkc)
                b0 = ib * PB
                ksl = slice(kc * G, (kc + 1) * G)
                den_blk = blk_state[ib]["den"]
                num_blk = blk_state[ib]["num"]
                nc.vector.reduce_sum(
                    out=num_blk[:, ksl], in_=state[ci]["e2"],
                    axis=mybir.AxisListType.X,
                )
                if kc == nkc - 1:
                    rden = perib.tile([PB, K], F32, tag="rden", name=f"rden{ib}")
                    nc.vector.reciprocal(out=rden, in_=den_blk)
                    mat_blk = perib.tile([PB, K], F32, tag="matblk", name=f"matblk{ib}")
                    nc.vector.tensor_mul(out=mat_blk, in0=num_blk, in1=rden)
                    nc.sync.dma_start(out=matcher[b0:b0 + PB, :], in_=mat_blk)
                state[ci].clear()

            state = [dict() for _ in range(NCI)]
            blk_state = {}
            stages = [s0, s1, s2, s3, s4]
            for tick in range(NCI + len(stages) - 1):
                for si in range(len(stages) - 1, -1, -1):
                    ci = tick - si
                    if 0 <= ci < NCI:
                        stages[si](ci)
    return nc


_NC_CACHE = None


def _host_consts(w, delays):
    aw = (ALPHA * w).astype(np.float32)
    decay_row = np.full((W,), DECAY, dtype=np.float32)
    decay_row[::T] = 0.0
    t_idx = np.arange(T, dtype=np.float64)
    lnt_seg = np.where(t_idx > 0, np.log(np.maximum(t_idx, 1e-30)), -1e4)
    lnt_row = np.tile(lnt_seg.astype(np.float32), G)
    ident_m = np.eye(PB, dtype=np.float32)
    ones1_v = np.ones((PB,), dtype=np.float32)
    return aw, decay_row, lnt_row, ident_m, ones1_v


def kernel(spike_times, w, delays):
    global _NC_CACHE
    if _NC_CACHE is None:
        _NC_CACHE = _build_nc()
        _split_multiwait(_NC_CACHE)
    nc = _NC_CACHE

    spike_times = np.ascontiguousarray(spike_times, dtype=np.float32)
    w = np.ascontiguousarray(w, dtype=np.float32)
    delays = np.ascontiguousarray(delays, dtype=np.float32)
    aw, decay_row, lnt_row, ident_m, ones1_v = _host_consts(w, delays)

    in_maps = []
    for c in range(N_CORES):
        in_maps.append({
            "spike": spike_times[c * BL:(c + 1) * BL],
            "delays": delays,
            "aw": aw,
            "decay": decay_row,
            "lnt": lnt_row,
            "ident": ident_m,
            "ones1": ones1_v,
        })

    res = run_bass_kernel_spmd(nc, in_maps, core_ids=list(range(N_CORES)))
    arrivals = np.concatenate([r["arrivals"] for r in res.results], axis=0)
    psp = np.concatenate([r["psp"] for r in res.results], axis=0)
    matcher = np.concatenate([r["matcher"] for r in res.results], axis=0)
    return arrivals, psp, matcher


# revision 34
# speedup vs baseline: 1.0126x; 1.0126x over previous
"""DendriticMatcherBank Trainium2 kernel (v3.1, streaming).

Math (per (b, k)):
    arrivals[b,k,t] = spike[b,t] + delays[k,t]
    tref[b,k]       = max_t arrivals
    psp             = exp((arrivals - tref)/40)
    I               = w * psp
    v_t             = 0.95 v_{t-1} + 0.05 I_t
    matcher[b,k]    = sum_t t*exp(v_t) / sum_t exp(v_t)  (softmax shift skipped: |v| <= ~1.5)

Sharding: B split across 8 cores (2048 rows each); (K,T) params replicated.

On-chip layout: partitions = 128 consecutive b; free = (k-chunk of G, t).

Engine assignment (per-core full passes over 16.8M elements):
  GPSIMD: arrivals = spike+delays;  I = (alpha w) * psp
  ACT:    psp = exp(arr/40 - tref/40) (per-k ops; bias is per-partition);
          e = exp(v); e2 = exp(v2)  (wide ops)
  DVE:    tref = rmax(arrivals); v = EMA scan; den = rsum(e); num = rsum(e2)
  PE:     v2 = v + ln(t) into PSUM (identity + rank-1 matmuls; PE idle otherwise)
The EMA runs as one tensor_tensor_scan along the flattened (g t) free dim,
with the decay operand zeroed at each segment start to reset between k's.
"""

import numpy as np

import concourse.bass as bass
import concourse.tile as tile
from concourse import mybir
from concourse.bass_utils import run_bass_kernel_spmd

B, K, T = 16384, 64, 128
N_CORES = 8
BL = B // N_CORES          # 2048 b-rows per core
PB = 128                   # b-rows per partition block
NB = BL // PB              # 16 b-blocks
G = 8                      # k's processed per chunk
NKC = K // G               # 8 k-chunks
W = G * T                  # 1024 free width per chunk
SLOT4 = 4                  # matmul slots per PSUM bank (4*T = 512 fp32 = 1 bank)
TAU_PSP = 40.0
ALPHA = 0.05               # DT / TAU_MEM
DECAY = 1.0 - ALPHA
F32 = mybir.dt.float32


def _split_multiwait(nc, max_waits=1):
    """Walrus in this env encodes at most one sync-wait per instruction; Tile's
    kernel-tail drain waits on every DMA-HW sem at once. Hoist extra waits onto
    single-wait NoOps inserted just before the offending instruction."""
    for fn in nc.m.functions:
        try:
            items = list(fn.blocks.items())
        except Exception:
            items = list(enumerate(fn.blocks))
        for _, blk in items:
            newlist = []
            changed = False
            for ins in blk.instructions:
                si = ins.sync_info
                waits = list(si.on_wait) if (si is not None and si.on_wait) else []
                if len(waits) > max_waits:
                    head, keep = waits[:-max_waits], waits[-max_waits:]
                    for wi, wt in enumerate(head):
                        nop = mybir.InstNoOp(name=f"{ins.name}-w{wi}", ins=[], outs=[])
                        nop.engine = ins.engine
                        nop.sync_info = mybir.SyncInfo(on_wait=[wt], on_update=[])
                        newlist.append(nop)
                    ins.sync_info = mybir.SyncInfo(
                        on_wait=keep, on_update=list(si.on_update or [])
                    )
                    changed = True
                newlist.append(ins)
            if changed:
                blk.instructions = newlist


def _pbcast(ap, p):
    """Broadcast an AP across p partitions (step-0 leading dim). DMA-only."""
    return bass.AP(tensor=ap.tensor, offset=ap.offset, ap=[[0, p]] + list(ap.ap))


def _build_nc(nb=NB, nkc=NKC, work_bufs=4, perib_bufs=2, stats_bufs=12, arr_bufs=6,
              psum_bufs=4, prio_s0=0):
    bl = nb * PB
    nc = bass.Bass("TRN2", num_devices=N_CORES)

    spike = nc.dram_tensor("spike", (bl, T), F32, kind="ExternalInput")
    delays = nc.dram_tensor("delays", (K, T), F32, kind="ExternalInput")
    aw = nc.dram_tensor("aw", (K, T), F32, kind="ExternalInput")        # ALPHA * w
    decay = nc.dram_tensor("decay", (W,), F32, kind="ExternalInput")    # 0.95, 0 at seg starts
    lnt = nc.dram_tensor("lnt", (W,), F32, kind="ExternalInput")        # ln(t) tiled, -1e4 at t=0
    ident = nc.dram_tensor("ident", (PB, PB), F32, kind="ExternalInput")
    ones1 = nc.dram_tensor("ones1", (PB,), F32, kind="ExternalInput")

    arrivals = nc.dram_tensor("arrivals", (bl, K, T), F32, kind="ExternalOutput")
    psp = nc.dram_tensor("psp", (bl, K, T), F32, kind="ExternalOutput")
    matcher = nc.dram_tensor("matcher", (bl, K), F32, kind="ExternalOutput")

    delays_f = delays.rearrange("k t -> (k t)")
    aw_f = aw.rearrange("k t -> (k t)")
    ngrp = G // SLOT4          # 4-slot matmul groups per chunk

    with tile.TileContext(nc) as tc:
        with (
            tc.tile_pool(name="consts", bufs=1) as consts,
            tc.tile_pool(name="spk", bufs=1) as spk,
            tc.tile_pool(name="perib", bufs=perib_bufs) as perib,
            tc.tile_pool(name="work", bufs=work_bufs) as work,
            tc.tile_pool(name="arrp", bufs=arr_bufs) as arrp,
            tc.tile_pool(name="stats", bufs=stats_bufs) as stats,
            tc.tile_pool(name="psB", bufs=psum_bufs, space="PSUM") as psB,
        ):
            # Load order tuned for fastest first-iteration start: the first
            # compute op needs spike[0] + dly chunk 0; ismul needs aw chunk 0.
            spike_t = [spk.tile([PB, T], F32, tag=f"spike{ib}", name=f"spike_t{ib}")
                       for ib in range(nb)]
            dly_bc = consts.tile([PB, K, T], F32, tag="dly")
            aw_bc = consts.tile([PB, K, T], F32, tag="aw")

            nc.sync.dma_start(out=spike_t[0], in_=spike[0:PB, :])
            nc.sync.dma_start(
                out=dly_bc[:, 0:G, :], in_=_pbcast(delays_f[0:G * T], PB),
            )
            decay_t = consts.tile([PB, W], F32, tag="decay")
            nc.sync.dma_start(out=decay_t, in_=_pbcast(decay.ap(), PB))
            nc.sync.dma_start(
                out=aw_bc[:, 0:G, :], in_=_pbcast(aw_f[0:G * T], PB),
            )
            ident_t = consts.tile([PB, PB], F32, tag="ident")
            nc.sync.dma_start(out=ident_t, in_=ident[:, :])
            ones1_t = consts.tile([1, PB], F32, tag="ones1")
            nc.sync.dma_start(out=ones1_t, in_=_pbcast(ones1.ap(), 1))
            lnt4_t = consts.tile([1, SLOT4 * T], F32, tag="lnt4")
            nc.sync.dma_start(out=lnt4_t, in_=_pbcast(lnt.ap()[:SLOT4 * T], 1))
            for kc in range(1, nkc):
                nc.sync.dma_start(
                    out=dly_bc[:, kc * G:(kc + 1) * G, :],
                    in_=_pbcast(delays_f[kc * G * T:(kc + 1) * G * T], PB),
                )
                nc.sync.dma_start(
                    out=aw_bc[:, kc * G:(kc + 1) * G, :],
                    in_=_pbcast(aw_f[kc * G * T:(kc + 1) * G * T], PB),
                )
            for ib in range(1, nb):
                nc.sync.dma_start(
                    out=spike_t[ib], in_=spike[ib * PB:(ib + 1) * PB, :]
                )

            NCI = nb * nkc

            def s0(ci):
                if prio_s0:
                    with tc.high_priority(offset=prio_s0):
                        return s0_body(ci)
                return s0_body(ci)

            def s0_body(ci):
                ib, kc = divmod(ci, nkc)
                b0 = ib * PB
                ksl = slice(kc * G, (kc + 1) * G)
                sap = spike_t[ib][:, :]
                spike_bc = bass.AP(
                    tensor=sap.tensor, offset=sap.offset,
                    ap=[sap.ap[0], [0, G], sap.ap[1]],
                )
                arr = arrp.tile([PB, G, T], F32, tag="arr", name=f"arr{ci}")
                nc.gpsimd.tensor_add(
                    out=arr, in0=spike_bc, in1=dly_bc[:, ksl, :]
                )
                nc.sync.dma_start(out=arrivals[b0:b0 + PB, ksl, :], in_=arr)
                tref_c = stats.tile([PB, G], F32, tag="trefc", name=f"trefc{ci}")
                nc.vector.reduce_max(
                    out=tref_c, in_=arr, axis=mybir.AxisListType.X, negate=True,
                )
                ntref = stats.tile([PB, G], F32, tag="ntref", name=f"ntref{ci}")
                nc.scalar.mul(out=ntref, in_=tref_c, mul=1.0 / TAU_PSP)
                state[ci]["arr"] = arr
                state[ci]["ntref"] = ntref

            def s1(ci):
                ib, kc = divmod(ci, nkc)
                b0 = ib * PB
                ksl = slice(kc * G, (kc + 1) * G)
                arr = state[ci]["arr"]
                ntref = state[ci]["ntref"]
                psp_t = work.tile([PB, G, T], F32, tag="psp", name=f"pspt{ci}")
                for j in range(G):
                    nc.scalar.activation(
                        out=psp_t[:, j, :], in_=arr[:, j, :],
                        func=mybir.ActivationFunctionType.Exp,
                        scale=1.0 / TAU_PSP,
                        bias=ntref[:, j:j + 1],
                    )
                nc.sync.dma_start(out=psp[b0:b0 + PB, ksl, :], in_=psp_t)
                i_s = work.tile([PB, G, T], F32, tag="is", name=f"is{ci}")
                nc.gpsimd.tensor_mul(out=i_s, in0=psp_t, in1=aw_bc[:, ksl, :])
                state[ci]["i_s"] = i_s

            def s2(ci):
                i_s = state[ci]["i_s"]
                v = work.tile([PB, G, T], F32, tag="v", name=f"v{ci}")
                nc.vector.tensor_tensor_scan(
                    out=v.rearrange("p g t -> p (g t)"),
                    data0=decay_t[:, :],
                    data1=i_s.rearrange("p g t -> p (g t)"),
                    initial=0.0,
                    op0=mybir.AluOpType.mult,
                    op1=mybir.AluOpType.add,
                )
                e = work.tile([PB, G, T], F32, tag="e", name=f"e{ci}")
                nc.scalar.activation(
                    out=e, in_=v, func=mybir.ActivationFunctionType.Exp,
                )
                v2p = psB.tile([PB, G, T], F32, tag="v2p", name=f"v2p{ci}")
                for g in range(ngrp):
                    sl = v2p[:, g * SLOT4:(g + 1) * SLOT4, :]
                    nc.tensor.matmul(
                        sl, ones1_t[:, :], lnt4_t[:1, :], start=True, stop=False,
                    )
                    nc.tensor.matmul(
                        sl, ident_t[:, :], v[:, g * SLOT4:(g + 1) * SLOT4, :],
                        start=False, stop=True,
                    )
                e2 = work.tile([PB, G, T], F32, tag="e2", name=f"e2{ci}")
                nc.scalar.activation(
                    out=e2, in_=v2p, func=mybir.ActivationFunctionType.Exp,
                )
                state[ci]["e"] = e
                state[ci]["e2"] = e2

            def s3(ci):
                ib, kc = divmod(ci, nkc)
                if kc == 0:
                    blk_state[ib] = {
                        "den": perib.tile([PB, K], F32, tag="denblk", name=f"denblk{ib}"),
                        "num": perib.tile([PB, K], F32, tag="numblk", name=f"numblk{ib}"),
                    }
                ksl = slice(kc * G, (kc + 1) * G)
                nc.vector.reduce_sum(
                    out=blk_state[ib]["den"][:, ksl], in_=state[ci]["e"],
                    axis=mybir.AxisListType.X,
                )

            def s4(ci):
                ib, kc = divmod(ci, nkc)
                b0 = ib * PB
                ksl = slice(kc * G, (kc + 1) * G)
                den_blk = blk_state[ib]["den"]
                num_blk = blk_state[ib]["num"]
                nc.vector.reduce_sum(
                    out=num_blk[:, ksl], in_=state[ci]["e2"],
                    axis=mybir.AxisListType.X,
                )
                if kc == nkc - 1:
                    rden = perib.tile([PB, K], F32, tag="rden", name=f"rden{ib}")
                    nc.vector.reciprocal(out=rden, in_=den_blk)
                    mat_blk = perib.tile([PB, K], F32, tag="matblk", name=f"matblk{ib}")
                    nc.vector.tensor_mul(out=mat_blk, in0=num_blk, in1=rden)
                    nc.sync.dma_start(out=matcher[b0:b0 + PB, :], in_=mat_blk)
                state[ci].clear()

            state = [dict() for _ in range(NCI)]
            blk_state = {}
            stages = [s0, s1, s2, s3, s4]
            for tick in range(NCI + len(stages) - 1):
                for si in range(len(stages) - 1, -1, -1):
                    ci = tick - si
                    if 0 <= ci < NCI:
                        stages[si](ci)
    return nc


_NC_CACHE = None


def _host_consts(w, delays):
    aw = (ALPHA * w).astype(np.float32)
    decay_row = np.full((W,), DECAY, dtype=np.float32)
    decay_row[::T] = 0.0
    t_idx = np.arange(T, dtype=np.float64)
    lnt_seg = np.where(t_idx > 0, np.log(np.maximum(t_idx, 1e-30)), -1e4)
    lnt_row = np.tile(lnt_seg.astype(np.float32), G)
    ident_m = np.eye(PB, dtype=np.float32)
    ones1_v = np.ones((PB,), dtype=np.float32)
    return aw, decay_row, lnt_row, ident_m, ones1_v


def kernel(spike_times, w, delays):
    global _NC_CACHE
    if _NC_CACHE is None:
        _NC_CACHE = _build_nc()
        _split_multiwait(_NC_CACHE)
    nc = _NC_CACHE

    spike_times = np.ascontiguousarray(spike_times, dtype=np.float32)
    w = np.ascontiguousarray(w, dtype=np.float32)
    delays = np.ascontiguousarray(delays, dtype=np.float32)
    aw, decay_row, lnt_row, ident_m, ones1_v = _host_consts(w, delays)

    in_maps = []
    for c in range(N_CORES):
        in_maps.append({
            "spike": spike_times[c * BL:(c + 1) * BL],
            "delays": delays,
            "aw": aw,
            "decay": decay_row,
            "lnt": lnt_row,
            "ident": ident_m,
            "ones1": ones1_v,
        })

    res = run_bass_kernel_spmd(nc, in_maps, core_ids=list(range(N_CORES)))
    arrivals = np.concatenate([r["arrivals"] for r in res.results], axis=0)
    psp = np.concatenate([r["psp"] for r in res.results], axis=0)
    matcher = np.concatenate([r["matcher"] for r in res.results], axis=0)
    return arrivals, psp, matcher
